# revision 40
# baseline (speedup 1.0000x reference)
"""Multi-head attention (B=8, S=1024, D=1024, H=16, dh=64) on 8 trn2 cores.

Sharding: data-parallel over batch — one batch element per NeuronCore, no
collectives. Per core the kernel computes, in fp16 with fp32 accumulation:

  K^T = Wk^T X^T, Q^T = Wq^T X^T  ([F on partitions, S free]; a head pair
                                   occupies partitions 0:64 / 64:128)
  V   = X Wv      [S part, F free] with a ones column per head (denominator)
  per head h:
    S^T = K_h Q_h^T        ([Sk part, Sq free]; K=64 contraction)
    E^T = exp(S^T / 8)     (ScalarE, fused scale, fp16 out)
    [O | d] = E^T.T [V_h|1]  (PSUM [Sq, 65]; col 64 = softmax denominator)
    out[:, h] = O * (1/d)  (VectorE reciprocal + per-partition scale)

Schedule: one fine-grained software pipeline over head pairs. Each
steady-state block hp interleaves, at score-row-tile (skm) granularity,
  - the K/Q projections of pair hp+1 (half a PSUM accumulation group per
    step),
  - the score matmuls + exp of pair hp,
  - the attention*V chains of pair hp-1 (one half-quad per step),
so the PE always has AV/projection work to run while ScalarE drains the
score PSUM banks, and ScalarE is fed continuously. The V projection is
woven into the first two blocks. The last pair is emitted head-by-head so
head 14's AV overlaps head 15's exp.

Host side only reshapes: slices the batch, transposes X to X^T and casts
fp32->fp16 (the rounding the on-chip matmuls would apply anyway).
"""

import numpy as np

S = 1024   # sequence length (queries == keys)
D = 1024   # model dim
F = 1024   # heads * head_dim
H = 16
DH = 64
P = 128
NCORES = 8
C = 68     # per-head column stride in the V buffer (64 vals + 1 ones + pad)
KD = D // P  # 8 contraction tiles

_cached_nc = None


def _build_nc():
    import concourse.tile as tile
    from concourse import bacc, mybir

    f32 = mybir.dt.float32
    f16 = mybir.dt.float16
    Exp = mybir.ActivationFunctionType.Exp

    nc = bacc.Bacc("TRN2", target_bir_lowering=False, debug=False,
                   num_devices=NCORES)

    xq_t = nc.dram_tensor("xq_t", [D, S], f16, kind="ExternalInput").ap()
    xk_t = nc.dram_tensor("xk_t", [D, S], f16, kind="ExternalInput").ap()
    xv_t = nc.dram_tensor("xv_t", [D, S], f16, kind="ExternalInput").ap()
    wq = nc.dram_tensor("wq", [D, F], f16, kind="ExternalInput").ap()
    wk = nc.dram_tensor("wk", [D, F], f16, kind="ExternalInput").ap()
    wv = nc.dram_tensor("wv", [D, F], f16, kind="ExternalInput").ap()
    out = nc.dram_tensor("out", [S, F], f32, kind="ExternalOutput").ap()

    with tile.TileContext(nc) as tc:
        with (
            tc.tile_pool(name="persist", bufs=1) as persist,
            tc.tile_pool(name="inputs", bufs=1) as inputs,
            tc.tile_pool(name="e_pool", bufs=6) as e_pool,
            tc.tile_pool(name="kq_ring", bufs=2) as kq_ring,
            tc.tile_pool(name="pout", bufs=2) as pout,
            tc.tile_pool(name="small", bufs=4) as small,
            tc.tile_pool(name="pp_ps", bufs=2, space="PSUM") as pp_ps,
            tc.tile_pool(name="s_ps", bufs=2, space="PSUM") as s_ps,
            tc.tile_pool(name="o_ps", bufs=2, space="PSUM") as o_ps,
        ):
            v65 = persist.tile([P, S // P, H * C], f16, tag="v65")
            v_heads = v65.rearrange("p s (h c) -> p s h c", c=C)
            nc.gpsimd.memset(v_heads[:, :, :, DH:DH + 1], 1.0)

            def load2(dram_x, dram_w, tag):
                tx = inputs.tile([P, KD, 1024], f16, tag=tag + "x")
                tw = inputs.tile([P, KD, 1024], f16, tag=tag + "w")
                for dc in range(KD):
                    nc.sync.dma_start(tx[:, dc, :],
                                      dram_x[dc * P:(dc + 1) * P, :])
                    nc.sync.dma_start(tw[:, dc, :],
                                      dram_w[dc * P:(dc + 1) * P, :])
                return tx, tw

            xk_sb, wk_sb = load2(xk_t, wk, "k")
            xq_sb, wq_sb = load2(xq_t, wq, "q")
            # xv/wv live in the E pool: their slots recycle into E tiles
            # once the V projection has consumed them
            xv_sb = e_pool.tile([P, KD, 1024], f16, tag="e", name="xv")
            wv_sb = e_pool.tile([P, KD, 1024], f16, tag="e", name="wv")
            for dc in range(KD):
                nc.sync.dma_start(xv_sb[:, dc, :], xv_t[dc * P:(dc + 1) * P, :])
                nc.sync.dma_start(wv_sb[:, dc, :], wv[dc * P:(dc + 1) * P, :])

            # ---- building blocks, emitted in interleaved order ----

            # Q is staged zero-padded, one buffer per head: head a's Q in
            # partitions 0:64 of qa (64:128 forever zero), head b's in
            # 64:128 of qb. Score matmuls can then contract over the full
            # 128 partitions (the other head's K meets exact zeros), so
            # every matmul in the kernel runs the same (128,128) PE tile
            # config — switching between 64-row and 128-row configs costs
            # ~100-300ns of pipeline refill per switch.
            qa_bufs = [kq_ring.tile([P, S], f16, tag=f"qa{r}", name=f"qa{r}",
                                    bufs=1) for r in range(2)]
            qb_bufs = [kq_ring.tile([P, S], f16, tag=f"qb{r}", name=f"qb{r}",
                                    bufs=1) for r in range(2)]
            for r in range(2):
                nc.gpsimd.memset(qa_bufs[r][DH:P, :], 0.0)
                nc.gpsimd.memset(qb_bufs[r][0:DH, :], 0.0)

            # K/Q projection for pair m: 4 PSUM groups (K j0, K j1, Q j0,
            # Q j1), each 8 contraction matmuls + a cast to SBUF, emitted
            # as half-groups (4 matmuls) so it spreads over a block.
            def make_kq_state(m):
                kc = kq_ring.tile([P, S], f16, tag="kc", name=f"kc{m}")
                return {"m": m, "kc": kc, "qa": qa_bufs[m % 2],
                        "qb": qb_bufs[m % 2], "ps": None}

            def kq_step(st, g, half):
                # g in 0..3: K j0, K j1, Q j0, Q j1; half in 0..1
                m = st["m"]
                j = g % 2
                lhs_sb, rhs_sb = (wk_sb, xk_sb) if g < 2 else (wq_sb, xq_sb)
                if half == 0:
                    st["ps"] = pp_ps.tile([P, 512], f32, tag="pp",
                                          name=f"pp{m}_{g}")
                ps = st["ps"]
                for dc in range(half * 4, half * 4 + 4):
                    nc.tensor.matmul(
                        ps[:, :],
                        lhsT=lhs_sb[:, dc, m * P:(m + 1) * P],
                        rhs=rhs_sb[:, dc, j * 512:(j + 1) * 512],
                        start=(dc == 0), stop=(dc == KD - 1),
                    )
                if half == 1:
                    cols = slice(j * 512, (j + 1) * 512)
                    if g < 2:
                        nc.vector.tensor_copy(st["kc"][:, cols], ps[:, :])
                    else:
                        nc.vector.tensor_copy(st["qa"][0:DH, cols],
                                              ps[0:DH, :])
                        nc.vector.tensor_copy(st["qb"][DH:P, cols],
                                              ps[DH:P, :])

            # V projection: 16 PSUM groups (m 0..7 x j 0..1) emitted as
            # 32 half-groups woven into the first two blocks.
            _vps = {}

            def v_step(m, j, half):
                key = (m, j)
                if half == 0:
                    _vps[key] = o_ps.tile([P, 512], f32, tag="o",
                                          name=f"vps{m}_{j}")
                ps = _vps[key]
                for dc in range(half * 4, half * 4 + 4):
                    nc.tensor.matmul(
                        ps[:, :],
                        lhsT=xv_sb[:, dc, m * P:(m + 1) * P],
                        rhs=wv_sb[:, dc, j * 512:(j + 1) * 512],
                        start=(dc == 0), stop=(dc == KD - 1),
                    )
                if half == 1:
                    src = ps.rearrange("p (h c) -> p h c", c=DH)
                    dst = v_heads[:, m, j * 8:(j + 1) * 8, 0:DH]
                    nc.vector.tensor_copy(dst, src)
                    del _vps[key]

            def make_es(hp):
                return {i: e_pool.tile([P, S // P, S], f16, tag="e",
                                       name=f"e{hp}_{i}")
                        for i in (0, 1)}

            # one skm step of scores+exp for pair hp; full-128 contraction
            # against the zero-padded Q buffers (all matmuls same config)
            def scores_skm(st, es, skm, heads=(0, 1)):
                kc = st["kc"]
                qp = {0: st["qa"], 1: st["qb"]}
                pss = {}
                # alternate allocation order per skm so each head's PSUM
                # rotates through both slots (ring depth 2 per head): the
                # skm+1 score matmuls then wait on the skm-1 exp, not skm's
                order = heads if skm % 2 == 0 else tuple(reversed(heads))
                for i in order:
                    pss[i] = s_ps.tile([P, S], f32, tag="s", name=f"s{i}")
                for j in range(2):
                    for i in heads:
                        nc.tensor.matmul(
                            pss[i][:, j * 512:(j + 1) * 512],
                            lhsT=kc[:, skm * P:(skm + 1) * P],
                            rhs=qp[i][:, j * 512:(j + 1) * 512],
                            start=True, stop=True,
                        )
                for i in heads:
                    nc.scalar.activation(es[i][:, skm, :], pss[i][:, :],
                                         Exp, scale=0.125)

            # AV for a pair: chains (sqm, head), processed in quads of 4
            # chains sharing one PSUM bank; a quad is emitted in two
            # half-quads so other work interleaves. After a quad: copy
            # out + reciprocal + scale (+ out DMA when flushing).
            def make_av_state(hp, es, heads=(0, 1), po=None, flush=True,
                              dma_engines=None):
                if po is None:
                    po = pout.tile([P, S // P, P], f32, tag="po",
                                   name=f"po{hp}_{min(heads)}")
                chains = [(sqm, i) for sqm in range(8) for i in heads]
                return {"hp": hp, "es": es, "heads": heads, "po": po,
                        "flush": flush, "ps_o": None, "chains": chains,
                        "dma_engines": dma_engines or [nc.sync]}

            def av_quad_half(stt, q, half):
                hp, es, po = stt["hp"], stt["es"], stt["po"]
                quad = stt["chains"][q * 4:(q + 1) * 4]
                if half == 0:
                    stt["ps_o"] = o_ps.tile([P, 512], f32, tag="o",
                                            name=f"avq{hp}_{q}_{min(stt['heads'])}")
                ps_o = stt["ps_o"]
                for c in range(half * 2, half * 2 + 2):
                    sqm, i = quad[c]
                    h = 2 * hp + i
                    for kt in range(8):
                        nc.tensor.matmul(
                            ps_o[:, c * P:c * P + DH + 1],
                            lhsT=es[i][:, kt, sqm * P:(sqm + 1) * P],
                            rhs=v65[:, kt, h * C:h * C + DH + 1],
                            start=(c == 0 and kt == 0),
                            stop=(c == 3 and kt == 7),
                            skip_group_check=True,
                        )
                if half == 0:
                    return
                st = small.tile([P, 4, DH + 1], f32, tag="st")
                src_v = ps_o.rearrange("p (c x) -> p c x", x=P)
                nc.vector.tensor_copy(st[:, 0:4, :], src_v[:, 0:4, 0:DH + 1])
                rt = small.tile([P, 4, 1], f32, tag="r")
                nc.vector.reciprocal(rt[:, 0:4, :], st[:, 0:4, DH:DH + 1])
                done = set()
                for c, (sqm, i) in enumerate(quad):
                    nc.vector.tensor_scalar_mul(
                        po[:, sqm, i * DH:(i + 1) * DH],
                        st[:, c, 0:DH], rt[:, c, :])
                    done.add(sqm)
                if stt["flush"]:
                    for sqm in sorted(done):
                        eng = stt["dma_engines"][sqm % len(stt["dma_engines"])]
                        eng.dma_start(
                            out[sqm * P:(sqm + 1) * P, hp * P:(hp + 1) * P],
                            po[:, sqm, :])

            # two projection groups emitted dc-interleaved: during the
            # input-DMA window the PE consumes each contraction chunk the
            # moment it lands instead of stalling on group A's later
            # chunks while group B's ready work waits behind it in
            # program order
            def kq_pair_dc(stA, gA, stB, gB):
                ps = {}
                for (st, g) in ((stA, gA), (stB, gB)):
                    ps[(st["m"], g)] = pp_ps.tile(
                        [P, 512], f32, tag="pp", name=f"pp{st['m']}_{g}")
                for dc in range(KD):
                    for (st, g) in ((stA, gA), (stB, gB)):
                        j = g % 2
                        lhs_sb, rhs_sb = ((wk_sb, xk_sb) if g < 2
                                          else (wq_sb, xq_sb))
                        nc.tensor.matmul(
                            ps[(st["m"], g)][:, :],
                            lhsT=lhs_sb[:, dc, st["m"] * P:(st["m"] + 1) * P],
                            rhs=rhs_sb[:, dc, j * 512:(j + 1) * 512],
                            start=(dc == 0), stop=(dc == KD - 1),
                        )
                for (st, g) in ((stA, gA), (stB, gB)):
                    j = g % 2
                    cols = slice(j * 512, (j + 1) * 512)
                    p = ps[(st["m"], g)]
                    if g < 2:
                        nc.vector.tensor_copy(st["kc"][:, cols], p[:, :])
                    else:
                        nc.vector.tensor_copy(st["qa"][0:DH, cols],
                                              p[0:DH, :])
                        nc.vector.tensor_copy(st["qb"][DH:P, cols],
                                              p[DH:P, :])

            # ---- the pipeline ----
            # prologue: K projections of pairs 0 AND 1 first (their input
            # lands first), then Q of pair 0, each pair of PSUM groups
            # dc-interleaved to match chunk arrival. Q of pair 1 is woven
            # into block 0's early steps.
            kq_states = {0: make_kq_state(0), 1: make_kq_state(1)}
            kq_pair_dc(kq_states[0], 0, kq_states[1], 0)
            kq_pair_dc(kq_states[0], 1, kq_states[1], 1)
            kq_pair_dc(kq_states[0], 2, kq_states[0], 3)

            # V weave: 32 half-groups over blocks 0-1 (two per step)
            v_halves = [(m, j, half)
                        for m in range(8) for j in range(2)
                        for half in range(2)]

            es_all = {}
            av_states = {}

            for hp in range(8):
                last = hp == 7
                heads = (0,) if last else (0, 1)
                es_all[hp] = make_es(hp)
                if not last and hp >= 1:
                    kq_states[hp + 1] = make_kq_state(hp + 1)
                # AV lags scores by TWO blocks: all V-weave emission (and
                # its DVE copies) strictly precedes any av(0) read of v65.
                if hp >= 2:
                    av_states[hp - 2] = make_av_state(hp - 2, es_all[hp - 2])
                for s in range(8):
                    # projections of next pair: one half-group per step
                    # (block 0 only has pair 1's Q left to do)
                    if hp == 0:
                        if s < 4:
                            g, half = 2 + s // 2, s % 2
                            kq_step(kq_states[1], g, half)
                    elif not last:
                        g, half = divmod(s, 2)
                        kq_step(kq_states[hp + 1], g, half)
                    # scores + exp of this pair
                    scores_skm(kq_states[hp], es_all[hp], s, heads=heads)
                    # V projection weave: two half-groups per step, after
                    # the scores so a DMA-stalled V group never blocks the
                    # score matmuls in PE program order
                    if hp < 2:
                        for k in range(2):
                            v_step(*v_halves[hp * 16 + s * 2 + k])
                    # AV two pairs back: one half-quad per step
                    if hp >= 2:
                        q, half = divmod(s, 2)
                        av_quad_half(av_states[hp - 2], q, half)

            # tail A: pair 7 head 1's scores+exp, interleaved with av(6)
            # and pair 7 head 0's AV
            po_last = pout.tile([P, S // P, P], f32, tag="po", name="polast")
            av6 = make_av_state(6, es_all[6])
            av7a = make_av_state(7, es_all[7], heads=(0,), po=po_last,
                                 flush=False)
            for s in range(8):
                scores_skm(kq_states[7], es_all[7], s, heads=(1,))
                q, half = divmod(s, 2)
                av_quad_half(av6, q, half)
                if s >= 4:  # head 0: 8 chains -> 2 quads over steps 4-7
                    q, half = divmod(s - 4, 2)
                    av_quad_half(av7a, q, half)
            # tail B: pair 7 head 1's AV + final output flush, spread
            # across four engine queues (all idle by now) so the last
            # eight descriptor generations don't serialize on Sync
            av7b = make_av_state(7, es_all[7], heads=(1,), po=po_last,
                                 flush=True,
                                 dma_engines=[nc.sync, nc.gpsimd,
                                              nc.scalar])
            for q in range(2):
                for half in range(2):
                    av_quad_half(av7b, q, half)

    nc.compile()
    return nc


def _get_nc():
    global _cached_nc
    if _cached_nc is None:
        _cached_nc = _build_nc()
    return _cached_nc


def _in_maps(queries, keys, values, Wq, Wk, Wv):
    f16 = np.float16
    wqb = np.ascontiguousarray(Wq).astype(f16)
    wkb = np.ascontiguousarray(Wk).astype(f16)
    wvb = np.ascontiguousarray(Wv).astype(f16)
    maps = []
    for b in range(NCORES):
        maps.append({
            "xq_t": queries[b].T.astype(f16),
            "xk_t": keys[b].T.astype(f16),
            "xv_t": values[b].T.astype(f16),
            "wq": wqb, "wk": wkb, "wv": wvb,
        })
    return maps


def kernel(queries, keys, values, Wq, Wk, Wv, _trace=False):
    from concourse import bass_utils

    queries = np.asarray(queries)
    keys = np.asarray(keys)
    values = np.asarray(values)
    Wq, Wk, Wv = np.asarray(Wq), np.asarray(Wk), np.asarray(Wv)
    nc = _get_nc()
    maps = _in_maps(queries, keys, values, Wq, Wk, Wv)
    res = bass_utils.run_bass_kernel_spmd(
        nc, maps, core_ids=list(range(NCORES)), trace=_trace)
    out = np.stack([res.results[b]["out"] for b in range(NCORES)])
    if _trace:
        kernel.last_results = res
    return out
